# revision 8
# baseline (speedup 1.0000x reference)
"""Trainium2 8-core Bass kernel: out = sigmoid(encoder_outputs @ hidden),
encoder_outputs [32768, 1024] f32, hidden [1024] f32 -> [1, 1, 32768] f32.

Sharding: encoder_outputs splits along seq_len into 8 slices of [4096, 1024]
(one per NeuronCore); hidden is replicated; each core produces its 4096
sigmoid scores and the host concatenates. No collectives needed.

Per-core kernel (raw bacc, hand-placed semaphores; no Tile machinery):
  - partition p owns rows [32p, 32p+32) of the slice; row r of a partition
    maps to scores/sig column r, so stores are contiguous per partition
  - hybrid stream start: Sync/HWDGE issues hidden + the first 2 rows as f32
    loads before the SWDGE ring init finishes, engaging HBM early
  - the remaining 30 rows stream as SWDGE cast-DMAs (f32 DRAM -> bf16
    SBUF); loads are small at both ends (fast first arrival, tight tail)
    and the final row arrives as two half-row loads so the last-byte ->
    stored-output chain is short
  - per load one DVE tensor_tensor multiplies all its rows against hidden
    (bf16, 2x packed); row sums are split between DVE and ACT by a static
    owner table: DVE rows use a folded reduce (bf16 pair-add at 2x into
    alternating fold buffers, then a 512-wide reduce — ~0.9us/row vs 1.3
    direct), ACT rows use activation(Copy)+accumulate on the product.
    DVE only reduces the first 1-2 rows of a load (the oldest-written
    product region) so no engine reads SBUF that was written less than
    ~1000 cycles earlier — raw bass has no same-engine RAW interlock.
  - row 31's two half-products are reduced by ACT's accumulator into two
    per-partition scalars and combined by the final activation's bias
    operand (sigmoid(a+b)), keeping the whole tail chain ACT-internal
  - a warm Sigmoid on the const-zero AP makes the single ACT funcset load
    happen at start, off the tail; sigmoid+store split 31/1 so only one
    column's worth of work follows the last arrival
  - branch hints arm the prefetcher for each engine's end-block branch;
    the end barrier is the cheap sem-only variant and the final store is
    gated by an explicit semaphore wait on Sync
Memory-bound at the ~430 GB/s/core HBM read roofline; bf16 multiply keeps
rel err ~6e-3 (gate 2e-2).
"""
import numpy as np
from concourse.bass_utils import run_bass_kernel_spmd


import concourse.bass as bass
from concourse import bacc, mybir


class _HintedBlock(bass.BassBlock):
    """no_gpsimd_drain block whose end-bb branches carry prefetch hints."""

    def __init__(self, bass_, name):
        super().__init__(bass_, name, no_gpsimd_drain=True)
        self.hint_locs = {}

    def __exit__(self, exc_type, exc_val, exc_tb):
        if exc_type is not None:
            return
        for engine, last_body in self.last_body.items():
            with self.bass.body(last_body, parent=self.bass.cur_bb,
                                allow_existing_parent=True):
                br = engine.br(self.end_bb)
                loc = self.hint_locs.get(engine)
                if loc is not None:
                    br.branch_hint(loc)
        self.bass.switch_bb(self.end_bb)
        gpsimd_type = self.bass.gpsimd.engine
        for eng_type, eng in self.bass.engines.items():
            if eng_type == gpsimd_type:
                continue
            d = mybir.InstDrain(
                name=self.bass.get_next_instruction_name(),
                ins=[], outs=[], bass_is_fusable=False)
            d.engine = eng_type
            eng.add_instruction(d)
        self.bass.all_engine_barrier(sem_only=True)

N_CORES = 8
SEQ = 32768
D = 1024
ROWS = SEQ // N_CORES          # 4096
RPP = ROWS // 128              # 32
F32 = mybir.dt.float32
BF16 = mybir.dt.bfloat16

HEAD_ROWS = 2                  # rows loaded f32 via HWDGE at kernel start
# SWDGE loads cover rows HEAD_ROWS..30 (full rows); row 31 arrives as two
# half-row loads appended after these.
LOAD_SIZES = [1, 2, 3, 4, 4, 4, 4, 3, 2, 2]
# rows whose reduce runs on DVE (folded); the rest go to ACT's accumulator.
# Only the first 1-2 rows of a load may be DVE-owned (hazard-safe regions).
# Row 29 must be the last DVE row <= 30 (it gates sigmoid 1).
DVE_ROWS = (3, 5, 8, 12, 13, 16, 17, 20, 21, 24, 27, 29)
OUT_SPLIT = 31                 # sigmoid/store split column


def build(load_sizes=LOAD_SIZES, dve_rows=DVE_ROWS, out_split=OUT_SPLIT,
          head_rows=HEAD_ROWS):
    assert head_rows + sum(load_sizes) == RPP - 1
    n_loads = len(load_sizes)
    cum_rows = np.cumsum([head_rows] + list(load_sizes))  # row0 of each load
    last = RPP - 1                                        # the half-row row
    dve_rows = frozenset(dve_rows)

    nc = bacc.Bacc("TRN2", target_bir_lowering=False, debug=False,
                   num_devices=N_CORES)
    h_dram = nc.dram_tensor("hidden", [D], F32, kind="ExternalInput")
    e_dram = nc.dram_tensor("encoder_outputs", [ROWS, D], F32,
                            kind="ExternalInput")
    o_dram = nc.dram_tensor("out", [ROWS], F32, kind="ExternalOutput")
    e_view = e_dram.ap().rearrange("(p r) d -> p (r d)", p=128)
    o_view = o_dram.ap().rearrange("(p r) -> p r", p=128)

    eallf = nc.alloc_sbuf_tensor("eallf", [128, head_rows * D], F32)
    eall = nc.alloc_sbuf_tensor("eall", [128, (RPP - head_rows) * D], BF16)
    htf = nc.alloc_sbuf_tensor("htf", [128, D], F32)
    ht = nc.alloc_sbuf_tensor("ht", [128, D], BF16)
    prodf = nc.alloc_sbuf_tensor("prodf", [128, head_rows * D], BF16)
    prods = [nc.alloc_sbuf_tensor(f"prod{i}", [128, sz * D], BF16)
             for i, sz in enumerate(load_sizes)]
    prodh = nc.alloc_sbuf_tensor("prodh", [128, D], BF16)  # row-31 halves
    folds = [nc.alloc_sbuf_tensor(f"fold{j}", [128, D // 2], BF16)
             for j in range(2)]
    tmp31a = nc.alloc_sbuf_tensor("tmp31a", [128, 1], F32)
    tmp31b = nc.alloc_sbuf_tensor("tmp31b", [128, 1], F32)
    scores = nc.alloc_sbuf_tensor("scores", [128, RPP], F32)
    sig = nc.alloc_sbuf_tensor("sigout", [128, RPP], F32)

    h_sem = nc.alloc_semaphore("hld")
    hd_sems = [nc.alloc_semaphore(f"hd{j}") for j in range(head_rows)]
    ld_sems = [nc.alloc_semaphore(f"ld{i}") for i in range(n_loads + 2)]
    tt_sem = nc.alloc_semaphore("tt")        # DVE TT progress (for ACT)
    rd_sem = nc.alloc_semaphore("rd")        # DVE tail gates (for ACT)
    sig_sem = nc.alloc_semaphore("sg")
    outd_sem = nc.alloc_semaphore("outd")

    fold_flip = [0]

    def fold_reduce(v, src, col):
        """DVE folded row sum: src is a [128, D] bf16 product row."""
        fold = folds[fold_flip[0]]
        fold_flip[0] ^= 1
        v.tensor_tensor(
            out=fold.ap(),
            in0=src[:, :D // 2],
            in1=src[:, D // 2:],
            op=mybir.AluOpType.add,
        )
        return v.tensor_reduce(
            out=scores.ap()[:, col:col + 1],
            in_=fold.ap().unsqueeze(1),
            axis=mybir.AxisListType.X, op=mybir.AluOpType.add,
        )

    with _HintedBlock(nc, f"blk{nc.next_id()}") as block:

        @block.gpsimd
        def _(g: bass.BassEngine):
            block.hint_locs[g] = g.mark_branch_hint_location()
            for i, sz in enumerate(load_sizes):
                r0 = int(cum_rows[i])
                g.dma_start(
                    out=eall.ap()[:, (r0 - head_rows) * D:
                                  (r0 - head_rows + sz) * D],
                    in_=e_view[:, r0 * D:(r0 + sz) * D],
                ).then_inc(ld_sems[i], 16)
            # row 31 as two half-row loads
            for h in range(2):
                lo = last * D + h * (D // 2)
                g.dma_start(
                    out=eall.ap()[:, (last - head_rows) * D + h * (D // 2):
                                  (last - head_rows) * D + (h + 1) * (D // 2)],
                    in_=e_view[:, lo:lo + D // 2],
                ).then_inc(ld_sems[n_loads + h], 16)

        @block.vector
        def _(v: bass.BassEngine):
            block.hint_locs[v] = v.mark_branch_hint_location()
            v.wait_ge(h_sem, 16)
            v.tensor_copy(out=ht.ap(), in_=htf.ap())
            # head rows: f32 TT (bf16 product out), reduces owned by ACT
            v.wait_ge(hd_sems[0], 16)
            v.wait_ge(hd_sems[1], 16)
            v.tensor_tensor(
                out=prodf.ap().rearrange("p (r d) -> p r d", r=head_rows),
                in0=eallf.ap().rearrange("p (r d) -> p r d", r=head_rows),
                in1=htf.ap().unsqueeze(1).broadcast_to((128, head_rows, D)),
                op=mybir.AluOpType.mult,
            ).then_inc(tt_sem, 1)
            for i, sz in enumerate(load_sizes):
                r0 = int(cum_rows[i])
                v.wait_ge(ld_sems[i], 16)
                v.tensor_tensor(
                    out=prods[i].ap().rearrange("p (r d) -> p r d", r=sz),
                    in0=eall.ap()[:, (r0 - head_rows) * D:
                                  (r0 - head_rows + sz) * D]
                        .rearrange("p (r d) -> p r d", r=sz),
                    in1=ht.ap().unsqueeze(1).broadcast_to((128, sz, D)),
                    op=mybir.AluOpType.mult,
                ).then_inc(tt_sem, 1)
                for k in range(sz):
                    r = r0 + k
                    if r in dve_rows:
                        red = fold_reduce(
                            v, prods[i].ap()[:, k * D:(k + 1) * D], r)
                        if r == 29:
                            red.then_inc(rd_sem, 1)
            # row 31: two half-row TTs; ACT reduces them
            v.wait_ge(ld_sems[n_loads], 16)
            v.tensor_tensor(
                out=prodh.ap()[:, :D // 2],
                in0=eall.ap()[:, (last - head_rows) * D:
                              (last - head_rows) * D + D // 2],
                in1=ht.ap()[:, :D // 2],
                op=mybir.AluOpType.mult,
            ).then_inc(rd_sem, 1)
            v.wait_ge(ld_sems[n_loads + 1], 16)
            v.tensor_tensor(
                out=prodh.ap()[:, D // 2:],
                in0=eall.ap()[:, (last - head_rows) * D + D // 2:
                              (last - head_rows + 1) * D],
                in1=ht.ap()[:, D // 2:],
                op=mybir.AluOpType.mult,
            ).then_inc(rd_sem, 1)

        @block.scalar
        def _(s: bass.BassEngine):
            block.hint_locs[s] = s.mark_branch_hint_location()
            # warm the sigmoid funcset (covers Copy too) so the tail sigmoid
            # doesn't trigger a second ACT table load
            cz = nc.const_aps.scalar_like(0.0, sig.ap()[:, 0:1])
            s.activation(out=sig.ap()[:, 0:1], in_=cz,
                         func=mybir.ActivationFunctionType.Sigmoid)
            # head rows (TT #1)
            s.wait_ge(tt_sem, 1)
            for j in range(head_rows):
                s.activation(
                    out=prodf.ap()[:, j * D:(j + 1) * D],
                    in_=prodf.ap()[:, j * D:(j + 1) * D],
                    func=mybir.ActivationFunctionType.Copy,
                    accum_out=scores.ap()[:, j:j + 1],
                )
            for i, sz in enumerate(load_sizes):
                r0 = int(cum_rows[i])
                act_rows = [k for k in range(sz) if (r0 + k) not in dve_rows]
                if act_rows:
                    s.wait_ge(tt_sem, i + 2)
                for k in act_rows:
                    col = r0 + k
                    s.activation(
                        out=prods[i].ap()[:, k * D:(k + 1) * D],
                        in_=prods[i].ap()[:, k * D:(k + 1) * D],
                        func=mybir.ActivationFunctionType.Copy,
                        accum_out=scores.ap()[:, col:col + 1],
                    )
            s.wait_ge(rd_sem, 1)
            s.activation(
                out=sig.ap()[:, :out_split],
                in_=scores.ap()[:, :out_split],
                func=mybir.ActivationFunctionType.Sigmoid,
            ).then_inc(sig_sem, 1)
            # row 31: reduce the two half products into per-partition
            # scalars, then combine with the bias operand: sigmoid(a + b)
            s.wait_ge(rd_sem, 2)
            s.activation(
                out=prodh.ap()[:, :D // 2],
                in_=prodh.ap()[:, :D // 2],
                func=mybir.ActivationFunctionType.Copy,
                accum_out=tmp31a.ap(),
            )
            s.wait_ge(rd_sem, 3)
            s.activation(
                out=prodh.ap()[:, D // 2:],
                in_=prodh.ap()[:, D // 2:],
                func=mybir.ActivationFunctionType.Copy,
                accum_out=tmp31b.ap(),
            )
            s.activation(
                out=sig.ap()[:, out_split:],
                in_=tmp31b.ap(),
                func=mybir.ActivationFunctionType.Sigmoid,
                bias=tmp31a.ap(),
            ).then_inc(sig_sem, 1)

        @block.sync
        def _(sy: bass.BassEngine):
            block.hint_locs[sy] = sy.mark_branch_hint_location()
            # hidden first (TTs need it), then the f32 head rows — all HWDGE,
            # issued before the SWDGE ring finishes initializing
            sy.dma_start(
                out=htf.ap(),
                in_=h_dram.ap().unsqueeze(0).broadcast_to((128, D))
            ).then_inc(h_sem, 16)
            for j in range(head_rows):
                sy.dma_start(
                    out=eallf.ap()[:, j * D:(j + 1) * D],
                    in_=e_view[:, j * D:(j + 1) * D],
                ).then_inc(hd_sems[j], 16)
            sy.wait_ge(sig_sem, 1)
            sy.dma_start(out=o_view[:, :out_split],
                         in_=sig.ap()[:, :out_split]).then_inc(outd_sem, 16)
            sy.wait_ge(sig_sem, 2)
            with nc.allow_non_contiguous_dma(
                    reason="final store is one f32 per partition (512B)"):
                sy.dma_start(out=o_view[:, out_split:],
                             in_=sig.ap()[:, out_split:]
                             ).then_inc(outd_sem, 16)
            sy.wait_ge(outd_sem, 32)

    nc.compile()
    return nc


def make_in_maps(hidden, encoder_outputs):
    hidden = np.ascontiguousarray(np.asarray(hidden, dtype=np.float32))
    encoder_outputs = np.asarray(encoder_outputs, dtype=np.float32)
    return [
        {"hidden": hidden,
         "encoder_outputs": np.ascontiguousarray(
             encoder_outputs[i * ROWS:(i + 1) * ROWS])}
        for i in range(N_CORES)
    ]


_NC_CACHE = None


def _get_nc():
    global _NC_CACHE
    if _NC_CACHE is None:
        _NC_CACHE = build()
    return _NC_CACHE


def _make_in_maps(hidden, encoder_outputs):
    return make_in_maps(hidden, encoder_outputs)


def kernel(hidden, encoder_outputs):
    nc = _get_nc()
    in_maps = make_in_maps(hidden, encoder_outputs)
    res = run_bass_kernel_spmd(nc, in_maps, core_ids=list(range(N_CORES)))
    out = np.concatenate(
        [np.asarray(res.results[i]["out"]).reshape(-1) for i in range(N_CORES)])
    return out[None, None, :].astype(np.float32)
